# revision 15
# baseline (speedup 1.0000x reference)
"""Per-sample modulated conv2d (StyleGAN2-style Conv2dMod) on 8 trn2 NeuronCores,
computed via 1D Winograd F(2,3) along H (direct 3-tap conv along W).

Reference (fp32):
    scale[n,o] = (1+y[n,o]) * rsqrt(||W[o]||^2 * (1+y[n,o])^2 + 1e-8)
    out = conv2d(edge_pad(x), W) * scale[:, :, None, None]

F(2,3) along H: out row-pair [2ti, 2ti+1] = A^T [ (G w_h) .* (B^T d_h) ] where
d_h = 4 input rows 2ti..2ti+3 of the padded image.  1.5x fewer MACs than
direct: per core 192 matmuls of [128x128] @ [128x512] (98k PE cycles vs 147k).
2D Winograd (2.25x) was measured transform-bound: DVE/Pool run ~1.1/0.3
elem/lane/ns, so its 75k elem/lane of transforms exceed the PE win; the 1D
row transform is only ~17k elem/lane (4 ops of +-1 combos, contiguous rows).

Sharding (4x2): core (g, h) = samples {2g, 2g+1} x out-channels [256h, +256).
Matmul free dim 512 = 2 samples x 8 row-tiles x 32 cols.

Per-core pipeline (phase structure hides the input DMA):
  - DMA interleaves per ic-chunk: [x(icc), U(*,*,icc)] so matmuls start ~5us in
  - row transform V_b = B^T-combos of stride-2 row views (vector engine, bf16)
  - phase tih0: all 8 psum groups (occ x b) accumulate icc-outer, paced by DMA
  - phase tih1: V and U resident -> pure 218ns/MM stream
  - output transform (A^T over b) on the vector engine reading PSUM pairs,
    writing interleaved row pairs; demod scale via tensor_scalar; bf16 out
    DMA per (occ, tih) quarter, fp32 cast on host
Host folds: U = G-transform of weights (bf16), edge-pad + layout x, scale.
Whole-pipeline bf16 rel err ~7e-3 (validated vs fp32 reference in numpy).
"""

import os

import numpy as np

N, C_IN, H, W = 8, 512, 32, 32
C_OUT, K = 512, 3
EPS = 1e-08
HP, WP = H + 2, W + 2  # 34x34 edge-padded
SMP, OCC, ICC, B, KW = 2, 2, 4, 4, 3  # per-core: samples, oc/ic chunks, pts, taps
TI = 16  # row-tile count (32/2)
FREE = SMP * (TI // 2) * W  # 512 matmul free dim (half the tiles per phase)
NCORES = 8


def _build_bass():
    import concourse.bass as bass  # noqa: F401
    import concourse.mybir as mybir
    import concourse.tile as tile
    from concourse import bacc
    from concourse.tile_rust import add_dep_helper

    f32 = mybir.dt.float32
    bf16 = mybir.dt.bfloat16
    ADD = mybir.AluOpType.add
    SUB = mybir.AluOpType.subtract

    nc = bacc.Bacc("TRN2")

    # [ic%128, icc, smp, h, w] padded input
    xp_d = nc.dram_tensor("xp", [128, ICC, SMP, HP, WP], bf16, kind="ExternalInput")
    # [occ, icc, ic%128, b, kw, oc%128] H-transformed weights
    u_d = nc.dram_tensor("u", [OCC, ICC, 128, B, KW, 128], bf16, kind="ExternalInput")
    # [oc%128, occ, smp] demod scale
    sc_d = nc.dram_tensor("sc", [128, OCC, SMP], f32, kind="ExternalInput")
    # [occ, oc%128, smp, h, w] scaled output
    out_d = nc.dram_tensor("out", [OCC, 128, SMP, H, W], bf16, kind="ExternalOutput")

    WARM_MMS = int(os.environ.get("WINO_WARM_MMS", "56"))
    CONC = int(os.environ.get("WINO_DMA_CONC", "6"))

    with tile.TileContext(nc) as tc:
        with (
            tc.tile_pool(name="singles", bufs=1) as singles,
            tc.tile_pool(name="psum", bufs=8, space="PSUM") as psum,
            tc.tile_pool(name="tap", bufs=2) as tap,
        ):
            sc_s = singles.tile([128, OCC, SMP], f32, name="sc")
            nc.gpsimd.dma_start(out=sc_s, in_=sc_d[:])

            # PE warm-up during the initial DMA (HAM clock-gate release)
            if WARM_MMS:
                wdum = singles.tile([128, 128], bf16, name="wdum")
                nc.vector.memset(wdum, 0.0)
                warm_ps = psum.tile([128, FREE], f32, tag="ps", name="warm")
                for _ in range(WARM_MMS):
                    nc.tensor.matmul(
                        warm_ps[:32, :128], wdum[:, :32], wdum, start=True, stop=True
                    )

            # Paced DMA chain: at most CONC transfers in flight, arriving in
            # consumption order (per-icc: x pair, then that icc's U slices).
            dma_chain = []

            def chain_dma(out, in_):
                eng = (nc.sync, nc.scalar)[len(dma_chain) % 2]
                bi = eng.dma_start(out=out, in_=in_)
                i = len(dma_chain)
                if i >= CONC:
                    add_dep_helper(
                        bi.ins, dma_chain[i - CONC].ins, sync=True, reason="dma pacing"
                    )
                dma_chain.append(bi)

            xp_t = singles.tile([128, ICC, SMP, HP, WP], bf16, name="xp")
            u_t = {
                occ: singles.tile([128, ICC, B, KW, 128], bf16, name=f"u{occ}")
                for occ in range(OCC)
            }
            for icc in range(ICC):
                chain_dma(xp_t[:, icc], xp_d[:, icc])
                for occ in range(OCC):
                    chain_dma(u_t[occ][:, icc], u_d[occ, icc])

            # ---- row transform (vector engine, bf16) ----
            # V_b = B^T-row combos of stride-2 row views of the padded image:
            # b0 = R0-R2, b1 = R1+R2, b2 = R2-R1, b3 = R1-R3
            bt = [(0, 2, SUB), (1, 2, ADD), (2, 1, SUB), (1, 3, SUB)]
            v_t = singles.tile([128, B, ICC, SMP, TI, WP], bf16, name="vt")
            for icc in range(ICC):
                for b, (r0, r1, op) in enumerate(bt):
                    nc.vector.tensor_tensor(
                        v_t[:, b, icc],
                        xp_t[:, icc, :, r0 : r0 + 31 : 2, :],
                        xp_t[:, icc, :, r1 : r1 + 31 : 2, :],
                        op,
                    )

            # ---- matmuls + output transform ----
            oimg = singles.tile([128, SMP, H, W], bf16, name="oimg")
            oimg2 = singles.tile([128, OCC, SMP, H, W], bf16, name="oimg2")

            def moving(b, icc, tih, kw):
                return v_t[:, b, icc, :, 8 * tih : 8 * tih + 8, kw : kw + W]

            def out_transform(occ, tih, ps):
                # even rows: (M0+M1)+M2 ; odd rows: (M1-M2)-M3 ; then scale.
                # Ops may read at most ONE PSUM operand, so the scalar engine
                # first copies M0/M1 to SBUF (it is otherwise idle).
                r0 = 16 * tih
                c0 = tap.tile([128, FREE], f32, tag="c0", name="c0")
                nc.scalar.copy(c0, ps[0])
                c1 = tap.tile([128, FREE], f32, tag="c1", name="c1")
                nc.scalar.copy(c1, ps[1])
                e1 = tap.tile([128, FREE], f32, tag="e1", name="e1")
                nc.vector.tensor_tensor(e1, c0, ps[1], ADD)
                nc.vector.tensor_tensor(
                    oimg[:, :, r0 : r0 + 16 : 2, :], e1, ps[2], ADD
                )
                o1 = tap.tile([128, FREE], f32, tag="o1", name="o1")
                nc.vector.tensor_tensor(o1, c1, ps[2], SUB)
                nc.vector.tensor_tensor(
                    oimg[:, :, r0 + 1 : r0 + 16 : 2, :], o1, ps[3], SUB
                )
                for smp in range(SMP):
                    nc.vector.tensor_scalar_mul(
                        oimg2[:, occ, smp, r0 : r0 + 16, :],
                        oimg[:, smp, r0 : r0 + 16, :],
                        sc_s[:, occ, smp : smp + 1],
                    )
                nc.gpsimd.dma_start(
                    out=out_d[occ, :, :, r0 : r0 + 16, :],
                    in_=oimg2[:, occ, :, r0 : r0 + 16, :],
                )

            # phase tih0: 8 psum groups (occ x b) accumulate icc-outer so the
            # matmul stream is gated only on (x, U, V) of the current icc
            ps0 = {
                (occ, b): psum.tile([128, FREE], f32, tag="ps", name=f"p0{occ}{b}")
                for occ in range(OCC)
                for b in range(B)
            }
            for icc in range(ICC):
                for occ in range(OCC):
                    for b in range(B):
                        for kw in range(KW):
                            nc.tensor.matmul(
                                ps0[(occ, b)],
                                u_t[occ][:, icc, b, kw],
                                moving(b, icc, 0, kw),
                                start=(icc == 0 and kw == 0),
                                stop=(icc == ICC - 1 and kw == KW - 1),
                            )
            for occ in range(OCC):
                out_transform(occ, 0, [ps0[(occ, b)] for b in range(B)])

            # phase tih1: everything resident -> dense per-group accumulation
            for occ in range(OCC):
                ps1 = []
                for b in range(B):
                    p = psum.tile([128, FREE], f32, tag="ps", name=f"p1{occ}{b}")
                    for icc in range(ICC):
                        for kw in range(KW):
                            nc.tensor.matmul(
                                p,
                                u_t[occ][:, icc, b, kw],
                                moving(b, icc, 1, kw),
                                start=(icc == 0 and kw == 0),
                                stop=(icc == ICC - 1 and kw == KW - 1),
                            )
                    ps1.append(p)
                out_transform(occ, 1, ps1)

    nc.finalize()
    return nc


def _prep_host(x: np.ndarray, y: np.ndarray, weight: np.ndarray):
    """Returns per-core input maps (list of 8 dicts). All layout work in numpy."""
    import ml_dtypes

    s = y + 1.0  # [N, O]
    wsq = np.sum(weight * weight, axis=(1, 2, 3))  # [O]
    scale = s / np.sqrt(wsq[None, :] * (s * s) + EPS)  # [N, O]

    # U = G-transform of the 3 H-taps -> [b, O, I, kw]
    G = np.array([[1, 0, 0], [0.5, 0.5, 0.5], [0.5, -0.5, 0.5], [0, 0, 1]], np.float32)
    U = np.einsum("bk,oikl->boil", G, weight)  # [B, O, I, KW]

    u_h = []
    for h in range(2):
        Uh = U[:, 256 * h : 256 * h + 256, :, :]  # [B, 256, 512, KW]
        Uh = Uh.reshape(B, OCC, 128, ICC, 128, KW)  # [b, occ, oc_lo, icc, ic_p, kw]
        Uh = Uh.transpose(1, 3, 4, 0, 5, 2)  # [occ, icc, ic_p, b, kw, oc_lo]
        u_h.append(np.ascontiguousarray(Uh.astype(ml_dtypes.bfloat16)))

    xp = np.pad(x, ((0, 0), (0, 0), (1, 1), (1, 1)), mode="edge")  # [N, C, 34, 34]
    xp = xp.reshape(N, ICC, 128, HP, WP)  # [n, icc, ic_p, h, w]

    in_maps = []
    for core in range(NCORES):
        g, h = divmod(core, 2)
        xc = xp[2 * g : 2 * g + 2]  # [smp, icc, ic_p, h, w]
        xc = xc.transpose(2, 1, 0, 3, 4).reshape(128, ICC * SMP, HP, WP)
        scg = scale[2 * g : 2 * g + 2, 256 * h : 256 * h + 256]  # [smp, 256]
        scg = scg.reshape(SMP, OCC, 128).transpose(2, 1, 0)  # [oc_p, occ, smp]
        in_maps.append(
            {
                "xp": np.ascontiguousarray(xc.astype(ml_dtypes.bfloat16)),
                "u": u_h[h],
                "sc": np.ascontiguousarray(scg.astype(np.float32)),
            }
        )
    return in_maps


def _gather(results) -> np.ndarray:
    out = np.empty((N, C_OUT, H, W), np.float32)
    for core in range(NCORES):
        g, h = divmod(core, 2)
        r = np.asarray(results[core]["out"]).astype(np.float32)  # [occ,128,smp,h,w]
        for occ in range(OCC):
            for smp in range(SMP):
                out[2 * g + smp, 256 * h + 128 * occ : 256 * h + 128 * (occ + 1)] = r[
                    occ, :, smp
                ]
    return out


def kernel(x: np.ndarray, y: np.ndarray, weight: np.ndarray) -> np.ndarray:
    from concourse.bass_utils import run_bass_kernel_spmd

    x = np.asarray(x, dtype=np.float32)
    y = np.asarray(y, dtype=np.float32)
    weight = np.asarray(weight, dtype=np.float32)

    in_maps = _prep_host(x, y, weight)
    nc = _build_bass()
    results = run_bass_kernel_spmd(nc, in_maps, core_ids=list(range(NCORES))).results
    return _gather(results)


# revision 16
# speedup vs baseline: 1.0709x; 1.0709x over previous
"""Per-sample modulated conv2d (StyleGAN2-style Conv2dMod) on 8 trn2 NeuronCores,
computed via 1D Winograd F(2,3) along H (direct 3-tap conv along W).

Reference (fp32):
    scale[n,o] = (1+y[n,o]) * rsqrt(||W[o]||^2 * (1+y[n,o])^2 + 1e-8)
    out = conv2d(edge_pad(x), W) * scale[:, :, None, None]

F(2,3) along H: out row-pair [2ti, 2ti+1] = A^T [ (G w_h) .* (B^T d_h) ] where
d_h = 4 input rows 2ti..2ti+3 of the padded image.  1.5x fewer MACs than
direct: per core 192 matmuls of [128x128] @ [128x512] (98k PE cycles vs 147k).
2D Winograd (2.25x) was measured transform-bound: DVE/Pool run ~1.1/0.3
elem/lane/ns, so its 75k elem/lane of transforms exceed the PE win; the 1D
row transform is only ~17k elem/lane (4 ops of +-1 combos, contiguous rows).

Sharding (4x2): core (g, h) = samples {2g, 2g+1} x out-channels [256h, +256).
Matmul free dim 512 = 2 samples x 8 row-tiles x 32 cols.

Per-core pipeline (phase structure hides the input DMA):
  - DMA interleaves per ic-chunk: [x(icc), U(*,*,icc)] so matmuls start ~5us in
  - row transform V_b = B^T-combos of stride-2 row views (vector engine, bf16)
  - phase tih0: all 8 psum groups (occ x b) accumulate icc-outer, paced by DMA
  - phase tih1: V and U resident -> pure 218ns/MM stream
  - output transform (A^T over b) on the vector engine reading PSUM pairs,
    writing interleaved row pairs; demod scale via tensor_scalar; bf16 out
    DMA per (occ, tih) quarter, fp32 cast on host
Host folds: U = G-transform of weights (bf16), edge-pad + layout x, scale.
Whole-pipeline bf16 rel err ~7e-3 (validated vs fp32 reference in numpy).
"""

import os

import numpy as np

N, C_IN, H, W = 8, 512, 32, 32
C_OUT, K = 512, 3
EPS = 1e-08
HP, WP = H + 2, W + 2  # 34x34 edge-padded
SMP, OCC, ICC, B, KW = 2, 2, 4, 4, 3  # per-core: samples, oc/ic chunks, pts, taps
TI = 16  # row-tile count (32/2)
FREE = SMP * (TI // 2) * W  # 512 matmul free dim (half the tiles per phase)
NCORES = 8


def _build_bass():
    import concourse.bass as bass  # noqa: F401
    import concourse.mybir as mybir
    import concourse.tile as tile
    from concourse import bacc
    from concourse.tile_rust import add_dep_helper

    f32 = mybir.dt.float32
    bf16 = mybir.dt.bfloat16
    ADD = mybir.AluOpType.add
    SUB = mybir.AluOpType.subtract

    nc = bacc.Bacc("TRN2")

    # [ic%128, icc, smp, h, w] padded input
    xp_d = nc.dram_tensor("xp", [128, ICC, SMP, HP, WP], bf16, kind="ExternalInput")
    # [occ, icc, ic%128, b, kw, oc%128] H-transformed weights
    u_d = nc.dram_tensor("u", [OCC, ICC, 128, B, KW, 128], bf16, kind="ExternalInput")
    # [oc%128, occ, smp] demod scale
    sc_d = nc.dram_tensor("sc", [128, OCC, SMP], f32, kind="ExternalInput")
    # [occ, oc%128, smp, h, w] scaled output
    out_d = nc.dram_tensor("out", [OCC, 128, SMP, H, W], bf16, kind="ExternalOutput")

    WARM_MMS = int(os.environ.get("WINO_WARM_MMS", "28"))
    CONC = int(os.environ.get("WINO_DMA_CONC", "6"))

    with tile.TileContext(nc) as tc:
        with (
            tc.tile_pool(name="singles", bufs=1) as singles,
            tc.tile_pool(name="psum", bufs=8, space="PSUM") as psum,
            tc.tile_pool(name="tap", bufs=2) as tap,
        ):
            sc_s = singles.tile([128, OCC, SMP], f32, name="sc")
            nc.gpsimd.dma_start(out=sc_s, in_=sc_d[:])

            # PE warm-up during the initial DMA (HAM clock-gate release)
            if WARM_MMS:
                wdum = singles.tile([128, 128], bf16, name="wdum")
                nc.vector.memset(wdum, 0.0)
                warm_ps = psum.tile([128, FREE], f32, tag="ps", name="warm")
                for _ in range(WARM_MMS):
                    nc.tensor.matmul(
                        warm_ps[:32, :128], wdum[:, :32], wdum, start=True, stop=True
                    )

            # Paced DMA chain: at most CONC transfers in flight, arriving in
            # consumption order (per-icc: x pair, then that icc's U slices).
            dma_chain = []

            def chain_dma(out, in_):
                eng = (nc.sync, nc.gpsimd)[len(dma_chain) % 2]
                bi = eng.dma_start(out=out, in_=in_)
                i = len(dma_chain)
                if i >= CONC:
                    add_dep_helper(
                        bi.ins, dma_chain[i - CONC].ins, sync=True, reason="dma pacing"
                    )
                dma_chain.append(bi)

            xp_t = singles.tile([128, ICC, SMP, HP, WP], bf16, name="xp")
            u_t = {
                occ: singles.tile([128, ICC, B, KW, 128], bf16, name=f"u{occ}")
                for occ in range(OCC)
            }
            for icc in range(ICC):
                if icc == 0:
                    chain_dma(xp_t[:, 0, 0], xp_d[:, 0, 0])
                    chain_dma(xp_t[:, 0, 1], xp_d[:, 0, 1])
                else:
                    chain_dma(xp_t[:, icc], xp_d[:, icc])
                for occ in range(OCC):
                    chain_dma(u_t[occ][:, icc], u_d[occ, icc])

            # ---- row transform (vector engine, bf16) ----
            # V_b = B^T-row combos of stride-2 row views of the padded image:
            # b0 = R0-R2, b1 = R1+R2, b2 = R2-R1, b3 = R1-R3
            bt = [(0, 2, SUB), (1, 2, ADD), (2, 1, SUB), (1, 3, SUB)]
            v_t = singles.tile([128, B, ICC, SMP, TI, WP], bf16, name="vt")
            for icc in range(ICC):
                for b, (r0, r1, op) in enumerate(bt):
                    nc.vector.tensor_tensor(
                        v_t[:, b, icc],
                        xp_t[:, icc, :, r0 : r0 + 31 : 2, :],
                        xp_t[:, icc, :, r1 : r1 + 31 : 2, :],
                        op,
                    )

            # ---- matmuls + output transform ----
            oimg = singles.tile([128, SMP, H, W], bf16, name="oimg")
            oimg2 = singles.tile([128, OCC, SMP, H, W], bf16, name="oimg2")

            def moving(b, icc, tih, kw):
                return v_t[:, b, icc, :, 8 * tih : 8 * tih + 8, kw : kw + W]

            def out_transform(occ, tih, ps):
                # even rows: (M0+M1)+M2 ; odd rows: (M1-M2)-M3 ; then scale.
                # Ops may read at most ONE PSUM operand, so the scalar engine
                # first copies M0/M1 to SBUF (it is otherwise idle).
                r0 = 16 * tih
                c0 = tap.tile([128, FREE], f32, tag="c0", name="c0")
                nc.scalar.copy(c0, ps[0])
                c1 = tap.tile([128, FREE], f32, tag="c1", name="c1")
                nc.scalar.copy(c1, ps[1])
                e1 = tap.tile([128, FREE], f32, tag="e1", name="e1")
                nc.vector.tensor_tensor(e1, c0, ps[1], ADD)
                o1 = tap.tile([128, FREE], f32, tag="o1", name="o1")
                nc.vector.tensor_tensor(o1, c1, ps[2], SUB)
                nc.vector.tensor_tensor(
                    oimg[:, :, r0 : r0 + 16 : 2, :], e1, ps[2], ADD
                )
                for smp in range(SMP):
                    nc.vector.tensor_scalar_mul(
                        oimg2[:, occ, smp, r0 : r0 + 16 : 2, :],
                        oimg[:, smp, r0 : r0 + 16 : 2, :],
                        sc_s[:, occ, smp : smp + 1],
                    )
                nc.vector.tensor_tensor(
                    oimg[:, :, r0 + 1 : r0 + 16 : 2, :], o1, ps[3], SUB
                )
                for smp in range(SMP):
                    nc.vector.tensor_scalar_mul(
                        oimg2[:, occ, smp, r0 + 1 : r0 + 16 : 2, :],
                        oimg[:, smp, r0 + 1 : r0 + 16 : 2, :],
                        sc_s[:, occ, smp : smp + 1],
                    )
                nc.gpsimd.dma_start(
                    out=out_d[occ, :, :, r0 : r0 + 16, :],
                    in_=oimg2[:, occ, :, r0 : r0 + 16, :],
                )

            # phase tih0: 8 psum groups (occ x b) accumulate icc-outer so the
            # matmul stream is gated only on (x, U, V) of the current icc
            ps0 = {
                (occ, b): psum.tile([128, FREE], f32, tag="ps", name=f"p0{occ}{b}")
                for occ in range(OCC)
                for b in range(B)
            }
            for icc in range(ICC):
                for occ in range(OCC):
                    for b in range(B):
                        for kw in range(KW):
                            nc.tensor.matmul(
                                ps0[(occ, b)],
                                u_t[occ][:, icc, b, kw],
                                moving(b, icc, 0, kw),
                                start=(icc == 0 and kw == 0),
                                stop=(icc == ICC - 1 and kw == KW - 1),
                            )
            for occ in range(OCC):
                out_transform(occ, 0, [ps0[(occ, b)] for b in range(B)])

            # phase tih1: everything resident -> dense per-group accumulation
            for occ in range(OCC):
                ps1 = []
                for b in range(B):
                    p = psum.tile([128, FREE], f32, tag="ps", name=f"p1{occ}{b}")
                    for icc in range(ICC):
                        for kw in range(KW):
                            nc.tensor.matmul(
                                p,
                                u_t[occ][:, icc, b, kw],
                                moving(b, icc, 1, kw),
                                start=(icc == 0 and kw == 0),
                                stop=(icc == ICC - 1 and kw == KW - 1),
                            )
                    ps1.append(p)
                out_transform(occ, 1, ps1)

    nc.finalize()
    return nc


def _prep_host(x: np.ndarray, y: np.ndarray, weight: np.ndarray):
    """Returns per-core input maps (list of 8 dicts). All layout work in numpy."""
    import ml_dtypes

    s = y + 1.0  # [N, O]
    wsq = np.sum(weight * weight, axis=(1, 2, 3))  # [O]
    scale = s / np.sqrt(wsq[None, :] * (s * s) + EPS)  # [N, O]

    # U = G-transform of the 3 H-taps -> [b, O, I, kw]
    G = np.array([[1, 0, 0], [0.5, 0.5, 0.5], [0.5, -0.5, 0.5], [0, 0, 1]], np.float32)
    U = np.einsum("bk,oikl->boil", G, weight)  # [B, O, I, KW]

    u_h = []
    for h in range(2):
        Uh = U[:, 256 * h : 256 * h + 256, :, :]  # [B, 256, 512, KW]
        Uh = Uh.reshape(B, OCC, 128, ICC, 128, KW)  # [b, occ, oc_lo, icc, ic_p, kw]
        Uh = Uh.transpose(1, 3, 4, 0, 5, 2)  # [occ, icc, ic_p, b, kw, oc_lo]
        u_h.append(np.ascontiguousarray(Uh.astype(ml_dtypes.bfloat16)))

    xp = np.pad(x, ((0, 0), (0, 0), (1, 1), (1, 1)), mode="edge")  # [N, C, 34, 34]
    xp = xp.reshape(N, ICC, 128, HP, WP)  # [n, icc, ic_p, h, w]

    in_maps = []
    for core in range(NCORES):
        g, h = divmod(core, 2)
        xc = xp[2 * g : 2 * g + 2]  # [smp, icc, ic_p, h, w]
        xc = xc.transpose(2, 1, 0, 3, 4).reshape(128, ICC * SMP, HP, WP)
        scg = scale[2 * g : 2 * g + 2, 256 * h : 256 * h + 256]  # [smp, 256]
        scg = scg.reshape(SMP, OCC, 128).transpose(2, 1, 0)  # [oc_p, occ, smp]
        in_maps.append(
            {
                "xp": np.ascontiguousarray(xc.astype(ml_dtypes.bfloat16)),
                "u": u_h[h],
                "sc": np.ascontiguousarray(scg.astype(np.float32)),
            }
        )
    return in_maps


def _gather(results) -> np.ndarray:
    out = np.empty((N, C_OUT, H, W), np.float32)
    for core in range(NCORES):
        g, h = divmod(core, 2)
        r = np.asarray(results[core]["out"]).astype(np.float32)  # [occ,128,smp,h,w]
        for occ in range(OCC):
            for smp in range(SMP):
                out[2 * g + smp, 256 * h + 128 * occ : 256 * h + 128 * (occ + 1)] = r[
                    occ, :, smp
                ]
    return out


def kernel(x: np.ndarray, y: np.ndarray, weight: np.ndarray) -> np.ndarray:
    from concourse.bass_utils import run_bass_kernel_spmd

    x = np.asarray(x, dtype=np.float32)
    y = np.asarray(y, dtype=np.float32)
    weight = np.asarray(weight, dtype=np.float32)

    in_maps = _prep_host(x, y, weight)
    nc = _build_bass()
    results = run_bass_kernel_spmd(nc, in_maps, core_ids=list(range(NCORES))).results
    return _gather(results)
